# revision 1
# baseline (speedup 1.0000x reference)
"""Trainium2 Bass kernel for a WaveNet-style dilated-conv stack.

Network (per reference):
  x1 = conv1d(x, Wc, bc, d=1, pad=1)                      # 1 -> 32, host-side (exact fp32)
  for l in 27 layers, d = 2^(l%9):
      g = tanh(conv(x, Wt_l, d)) * sigmoid(conv(x, Ws_l, d))   # 32->32, k=3, pad=d
      skip += conv1x1(g, Wskip_l)                              # 32->512
      x = conv1x1(g, Wdense_l) + x
  out = conv1x1(relu(conv1x1(skip, Wp1)), Wp2)            # 512->512->256
  return log_softmax(out, axis=channels)

Device strategy (8 cores, sequence-parallel):
  - Wp1 folded into skip weights on host: W1s_l = Wp1 @ Wskip_l, so
    h = Wp1@skip + bp1 is accumulated directly (512 ch), then relu -> Wp2.
  - Each core owns 16384 steps, processed as 2 halves of 8192 with a 1536-step
    halo (total receptive radius of the dilated stack is 1533).  Edge windows
    use zero/stale padding; contamination moves <= d per layer, so the valid
    region stays exact.  No cross-core communication.
  - g for 4 consecutive layers is staged in a [128, W] "ring" so the skip
    projection runs as single K=128 matmuls.  The dense 1x1 conv is a single
    K=128 matmul with zeros outside the layer's ring strip (this backend
    rejects PSUM accumulation across different PE row strips).
  - bf16 matmuls (fp32 PSUM), fp32 post-processing.
"""

import numpy as np
import ml_dtypes

BF16 = ml_dtypes.bfloat16

DIL = [2 ** i for i in range(9)] * 3
L = len(DIL)            # 27
RD, SD, QD = 32, 512, 256
T = 131072
NCORES = 8
V = T // NCORES         # 16384 per core
VH = V // 2             # 8192 per half
HALO = 1536             # >= 1533 total dilation radius
PAD = 256               # >= max dilation, so tap reads never go OOB
WH = VH + 2 * HALO      # 11264 computed window per half
WA = WH + 2 * PAD       # 11776 allocated width per half
VOFF = HALO + PAD       # 1792 valid-region offset inside the window
NGRP = (L + 3) // 4     # 7 groups of (up to) 4 layers for K=128 skip matmuls
ATILE = 1024            # activation tile width
NA = WH // ATILE        # 11 act tiles per layer per half
NB = VH // 512          # 16 valid 512-col blocks per half

_cache = {}
_last_run = {}


def _build():
    from contextlib import ExitStack

    import concourse.bacc as bacc
    import concourse.mybir as mybir
    import concourse.tile as tile

    dt = mybir.dt
    AF = mybir.ActivationFunctionType
    ALU = mybir.AluOpType
    f32, bf16 = dt.float32, dt.bfloat16

    nc = bacc.Bacc("TRN2", target_bir_lowering=False, debug=False,
                   num_devices=NCORES)

    def din(name, shape, dty):
        return nc.dram_tensor(name, shape, dty, kind="ExternalInput").ap()

    xin_d = din("xin", [RD, 2 * WA], bf16)
    wg_d = din("wg", [64, L * 3 * 64], bf16)       # gated lhsT, 2 strip replicas
    wdx_d = din("wdx", [128, L * RD], bf16)        # dense lhsT (strip-embedded)
    idw_d = din("idw", [128, 2 * RD], bf16)        # residual identity lhsT
    wskp_d = din("wskp", [128, NGRP * 4 * 128], bf16)  # skip lhsT per (grp, m)
    wp2_d = din("wp2", [128, 8 * 128], bf16)       # Wp2 lhsT per (q, p)
    bts_d = din("bts", [RD, L], f32)
    bss_d = din("bss", [RD, L], f32)
    bdc_d = din("bdc", [RD, L], f32)
    hb_d = din("hb", [128, 4], f32)
    bp2c_d = din("bp2c", [128, 2], f32)
    sumw_d = din("sumw", [128, 1], f32)
    nones_d = din("nones", [1, 128], f32)
    out_d = nc.dram_tensor("out", [QD, V], f32, kind="ExternalOutput").ap()

    with tile.TileContext(nc) as tc, ExitStack() as top:
        wp = top.enter_context(tc.tile_pool(name="wp", bufs=1))

        def load(d, tag):
            t = wp.tile(list(d.shape), d.dtype, tag=tag, name=tag)
            nc.sync.dma_start(t[:], d[:])
            return t

        wg = load(wg_d, "wg")
        wdx = load(wdx_d, "wdx")
        idw = load(idw_d, "idw")
        wskp = load(wskp_d, "wskp")
        wp2 = load(wp2_d, "wp2")
        bts = load(bts_d, "bts")
        bss = load(bss_d, "bss")
        bdc = load(bdc_d, "bdc")
        hb = load(hb_d, "hb")
        bp2c = load(bp2c_d, "bp2c")
        sumw = load(sumw_d, "sumw")
        nones = load(nones_d, "nones")

        # x ping-pongs between partition strips 0/1 of one [128, W] tensor so
        # the residual add runs on the PE as a K=128 identity matmul.
        xx = wp.tile([128, WA], bf16, tag="xx", name="xx")
        ring = wp.tile([128, WA], bf16, tag="ring", name="ring")
        h = wp.tile([128, 4 * VH], bf16, tag="h", name="h")
        nc.vector.memset(xx[:], 0.0)
        nc.vector.memset(ring[:], 0.0)

        for half in range(2):
            nc.sync.dma_start(xx[0:RD, :], xin_d[:, half * WA:(half + 1) * WA])
            with ExitStack() as lctx:
                pg = lctx.enter_context(
                    tc.tile_pool(name=f"pg{half}", bufs=3, space="PSUM"))
                pk = lctx.enter_context(
                    tc.tile_pool(name=f"pk{half}", bufs=2, space="PSUM"))
                tu = lctx.enter_context(tc.tile_pool(name=f"tu{half}", bufs=3))

                for l in range(L):
                    d = DIL[l]
                    j = l % 4
                    G = l // 4
                    sc = RD * (l % 2)        # strip of x_l
                    sn = RD * ((l + 1) % 2)  # strip of x_{l+1}
                    for a in range(NA):
                        b0 = PAD + a * ATILE
                        pgt = pg.tile([128, ATILE], f32, tag="pg", name="pg")
                        for s in range(2):
                            c0 = b0 + s * 512
                            for k in range(3):
                                nc.tensor.matmul(
                                    pgt[0:64, s * 512:(s + 1) * 512],
                                    wg[sc:sc + RD,
                                       (l * 3 + k) * 64:(l * 3 + k + 1) * 64],
                                    xx[sc:sc + RD, c0 + (k - 1) * d:
                                       c0 + (k - 1) * d + 512],
                                    start=(k == 0), stop=(k == 2),
                                    tile_position=(sc, 0))
                        tt = tu.tile([RD, ATILE], bf16, tag="t", name="t")
                        uu = tu.tile([RD, ATILE], bf16, tag="u", name="u")
                        nc.scalar.activation(tt[:], pgt[0:RD, :], AF.Tanh,
                                             bias=bts[:, l:l + 1])
                        nc.scalar.activation(uu[:], pgt[RD:2 * RD, :],
                                             AF.Sigmoid, bias=bss[:, l:l + 1])
                        nc.vector.tensor_mul(
                            ring[RD * j:RD * (j + 1), b0:b0 + ATILE],
                            tt[:], uu[:])
                        for s in range(2):
                            c0 = b0 + s * 512
                            pxs = pgt[64 + RD * s:96 + RD * s,
                                      s * 512:(s + 1) * 512]
                            nc.tensor.matmul(
                                pxs, wdx[:, l * RD:(l + 1) * RD],
                                ring[:, c0:c0 + 512], start=True, stop=False,
                                tile_position=(0, 64 + RD * s))
                            nc.tensor.matmul(
                                pxs, idw[:, RD * (l % 2):RD * (l % 2) + RD],
                                xx[:, c0:c0 + 512], start=False, stop=True,
                                tile_position=(0, 64 + RD * s))
                            # x_new = psum + bdense (residual already in psum)
                            nc.vector.tensor_scalar_add(
                                xx[sn:sn + RD, c0:c0 + 512], pxs,
                                bdc[:, l:l + 1])

                    if j == 3 or l == L - 1:
                        # skip contribution of this 4-layer group (K=128)
                        for m in range(4):
                            for cb in range(NB):
                                c0 = VOFF + cb * 512
                                pst = pk.tile([128, 512], f32, tag="pk",
                                              name="pk")
                                nc.tensor.matmul(
                                    pst[:],
                                    wskp[:, (G * 4 + m) * 128:
                                         (G * 4 + m + 1) * 128],
                                    ring[:, c0:c0 + 512],
                                    start=True, stop=True)
                                hcol = m * VH + cb * 512
                                if G == 0:
                                    nc.vector.tensor_copy(
                                        h[:, hcol:hcol + 512], pst[:])
                                else:
                                    nc.vector.tensor_add(
                                        h[:, hcol:hcol + 512],
                                        h[:, hcol:hcol + 512], pst[:])

            with ExitStack() as pctx:
                pop = pctx.enter_context(
                    tc.tile_pool(name=f"po{half}", bufs=4, space="PSUM"))
                psp = pctx.enter_context(
                    tc.tile_pool(name=f"ps{half}", bufs=2, space="PSUM"))
                pqp = pctx.enter_context(
                    tc.tile_pool(name=f"pq{half}", bufs=2, space="PSUM"))
                sp = pctx.enter_context(tc.tile_pool(name=f"sp{half}", bufs=2))
                for cb in range(NB):
                    rr = sp.tile([128, 4 * 512], bf16, tag="r", name="r")
                    for m in range(4):
                        nc.scalar.activation(
                            rr[:, m * 512:(m + 1) * 512],
                            h[:, m * VH + cb * 512:m * VH + cb * 512 + 512],
                            AF.Relu, bias=hb[:, m:m + 1])
                    pos = []
                    for p in range(2):
                        pot = pop.tile([128, 512], f32, tag="po", name="po")
                        for q in range(4):
                            nc.tensor.matmul(
                                pot[:],
                                wp2[:, (q * 2 + p) * 128:(q * 2 + p + 1) * 128],
                                rr[:, q * 512:(q + 1) * 512],
                                start=(q == 0), stop=(q == 3))
                        pos.append(pot)
                    ee = sp.tile([128, 1024], f32, tag="e", name="e")
                    for p in range(2):
                        nc.scalar.activation(ee[:, p * 512:(p + 1) * 512],
                                             pos[p][:], AF.Exp,
                                             bias=bp2c[:, p:p + 1])
                    pst = psp.tile([128, 512], f32, tag="ps", name="ps")
                    for p in range(2):
                        nc.tensor.matmul(pst[0:1, :], sumw[:],
                                         ee[:, p * 512:(p + 1) * 512],
                                         start=(p == 0), stop=(p == 1))
                    lss = sp.tile([1, 512], f32, tag="ls", name="ls")
                    nc.scalar.activation(lss[:], pst[0:1, :], AF.Ln)
                    pqt = pqp.tile([128, 512], f32, tag="pq", name="pq")
                    nc.tensor.matmul(pqt[:], nones[:], lss[:],
                                     start=True, stop=True)
                    oo = sp.tile([128, 1024], f32, tag="o", name="o")
                    oo2 = sp.tile([128, 1024], f32, tag="o2", name="o2")
                    for p in range(2):
                        nc.scalar.activation(oo[:, p * 512:(p + 1) * 512],
                                             pos[p][:], AF.Identity,
                                             bias=bp2c[:, p:p + 1])
                        nc.vector.tensor_add(oo2[:, p * 512:(p + 1) * 512],
                                             oo[:, p * 512:(p + 1) * 512],
                                             pqt[:])
                        c0 = half * VH + cb * 512
                        nc.sync.dma_start(
                            out_d[p * 128:(p + 1) * 128, c0:c0 + 512],
                            oo2[:, p * 512:(p + 1) * 512])

    nc.compile()
    return nc


def _prep_host(inputs):
    """Host-side exact fp32 preprocessing: initial conv, weight packing."""
    x = np.asarray(inputs["x"], np.float32)
    Wc = np.asarray(inputs["Wc"], np.float32)
    bc = np.asarray(inputs["bc"], np.float32)
    Wt = np.asarray(inputs["Wt"], np.float32)
    bt = np.asarray(inputs["bt"], np.float32)
    Ws = np.asarray(inputs["Ws"], np.float32)
    bs = np.asarray(inputs["bs"], np.float32)
    Wskip = np.asarray(inputs["Wskip"], np.float32)
    bskip = np.asarray(inputs["bskip"], np.float32)
    Wdense = np.asarray(inputs["Wdense"], np.float32)
    bdense = np.asarray(inputs["bdense"], np.float32)
    Wp1 = np.asarray(inputs["Wp1"], np.float32)
    bp1 = np.asarray(inputs["bp1"], np.float32)
    Wp2 = np.asarray(inputs["Wp2"], np.float32)
    bp2 = np.asarray(inputs["bp2"], np.float32)

    # initial conv (1 -> 32, k=3, pad=1), exact fp32 on host
    x0 = x[0, 0]
    xp = np.pad(x0, (1, 1))
    x1 = (Wc[:, 0, 0:1] * xp[None, 0:T]
          + Wc[:, 0, 1:2] * xp[None, 1:T + 1]
          + Wc[:, 0, 2:3] * xp[None, 2:T + 2]) + bc[:, None]
    xg = np.pad(x1, ((0, 0), (VOFF, VOFF)))

    xin = np.empty((NCORES, RD, 2 * WA), BF16)
    for c in range(NCORES):
        for hf in range(2):
            s = c * V + hf * VH
            xin[c, :, hf * WA:(hf + 1) * WA] = xg[:, s:s + WA].astype(BF16)

    wg = np.zeros((64, L * 3 * 64), np.float32)
    wdx = np.zeros((128, L * RD), np.float32)
    for l in range(L):
        for k in range(3):
            blk = np.concatenate([Wt[l, :, :, k].T, Ws[l, :, :, k].T], axis=1)
            for p in range(2):
                wg[RD * p:RD * (p + 1),
                   (l * 3 + k) * 64:(l * 3 + k + 1) * 64] = blk
        j = l % 4
        wdx[RD * j:RD * (j + 1), l * RD:(l + 1) * RD] = Wdense[l, :, :, 0].T

    idw = np.zeros((128, 2 * RD), np.float32)
    for p in range(2):
        idw[RD * p:RD * (p + 1), RD * p:RD * (p + 1)] = np.eye(RD)

    W1s = np.einsum("ab,lbc->lac", Wp1[:, :, 0], Wskip[:, :, :, 0])  # [L,512,32]
    wskp = np.zeros((128, NGRP * 4 * 128), np.float32)
    for G in range(NGRP):
        for m in range(4):
            for j in range(4):
                l = G * 4 + j
                if l < L:
                    wskp[32 * j:32 * (j + 1),
                         (G * 4 + m) * 128:(G * 4 + m + 1) * 128] = \
                        W1s[l, 128 * m:128 * (m + 1), :].T

    wp2 = np.zeros((128, 8 * 128), np.float32)
    for q in range(4):
        for p in range(2):
            wp2[:, (q * 2 + p) * 128:(q * 2 + p + 1) * 128] = \
                Wp2[128 * p:128 * (p + 1), 128 * q:128 * (q + 1), 0].T

    hbias = Wp1[:, :, 0] @ bskip.sum(axis=0) + bp1     # [512]
    hb = hbias.reshape(4, 128).T.copy()                # [128, 4]

    shared = {
        "wg": wg.astype(BF16),
        "wdx": wdx.astype(BF16),
        "idw": idw.astype(BF16),
        "wskp": wskp.astype(BF16),
        "wp2": wp2.astype(BF16),
        "bts": np.ascontiguousarray(bt.T.astype(np.float32)),
        "bss": np.ascontiguousarray(bs.T.astype(np.float32)),
        "bdc": np.ascontiguousarray(bdense.T.astype(np.float32)),
        "hb": np.ascontiguousarray(hb.astype(np.float32)),
        "bp2c": np.ascontiguousarray(bp2.reshape(2, 128).T.astype(np.float32)),
        "sumw": np.ones((128, 1), np.float32),
        "nones": np.full((1, 128), -1.0, np.float32),
    }
    return xin, shared


def kernel(**inputs):
    from concourse.bass_utils import run_bass_kernel_spmd

    xin, shared = _prep_host(inputs)
    if "nc" not in _cache:
        _cache["nc"] = _build()
    nc = _cache["nc"]

    in_maps = [dict(shared, xin=np.ascontiguousarray(xin[c]))
               for c in range(NCORES)]
    res = run_bass_kernel_spmd(nc, in_maps, core_ids=list(range(NCORES)))

    _last_run["nc"] = nc
    _last_run["in_maps"] = in_maps

    out = np.empty((1, QD, T), np.float32)
    for c in range(NCORES):
        out[0, :, c * V:(c + 1) * V] = res.results[c]["out"]
    return out



# revision 11
# speedup vs baseline: 1.0993x; 1.0993x over previous
"""Trainium2 Bass kernel for a WaveNet-style dilated-conv stack (v2).

Network (per reference):
  x1 = conv1d(x, Wc, bc, d=1, pad=1)                      # 1 -> 32, host-side (exact fp32)
  for l in 27 layers, d = 2^(l%9):
      g = tanh(conv(x, Wt_l, d)) * sigmoid(conv(x, Ws_l, d))   # 32->32, k=3, pad=d
      skip += conv1x1(g, Wskip_l)                              # 32->512
      x = conv1x1(g, Wdense_l) + x
  out = conv1x1(relu(conv1x1(skip, Wp1)), Wp2)            # 512->512->256
  return log_softmax(out, axis=channels)

Device strategy (8 cores, sequence-parallel, no cross-core comm):
  - Wp1 folded into skip weights on host (W1s = Wp1 @ Wskip).
  - Each core owns 16384 steps as 2 halves of 8192 with a 2048-step halo
    (receptive radius 1533 < 2048); window = 12288 computed cols = 6 units
    of 2048.
  - Gated convs: per 2048-unit, 4 column strips of 512 go to a single
    [128, 1024] PSUM tile via col-tiled M=32 matmuls (strip s -> psum
    partitions 32s; tanh at cols 0:512, sigmoid at 512:1024).  One tanh +
    one sigmoid activation per unit then run 128 lanes wide.
  - g = tanh*sig computed stacked [128, 512] then scattered to the flat
    ring (4x bf16 copies).
  - Two ring panels of 4 layers each; skip matmuls accumulate 2 groups
    (8 layers) per PSUM tile before one DVE add into h.
  - Residual + bias folded into the PSUM->SBUF x evacuation:
    even layers on DVE via scalar_tensor_tensor((pd+bd)+x_old),
    odd layers on ScalarE via Identity activation (residual via identity
    matmul on the PE).
  - Post stage: h layout [128, cb*2048+m*512] so relu is one op per cb;
    log-softmax broadcast done by accumulating -log(sum) into the output
    PSUM bank on the PE; final bias via ScalarE Identity.  No DVE in post.
"""

import numpy as np
import ml_dtypes

BF16 = ml_dtypes.bfloat16

DIL = [2 ** i for i in range(9)] * 3
L = len(DIL)            # 27
RD, SD, QD = 32, 512, 256
T = 131072
NCORES = 8
V = T // NCORES         # 16384 per core
VH = V // 2             # 8192 per half
HALO = 2048             # >= 1533 total dilation radius
PAD = 256               # >= max dilation, tap reads never go OOB
WH = VH + 2 * HALO      # 12288 computed window per half
WA = WH + 2 * PAD       # 12800 allocated width per half
VOFF = HALO + PAD       # 2304 valid-region offset inside window
NU = WH // 2048         # 6 units of 2048 per half
NB = VH // 512          # 16 valid 512-col blocks per half
NGRP = (L + 3) // 4     # 7 groups of (up to) 4 layers
PAIRS = [(0, 1), (2, 3), (4, 5), (6,)]

_cache = {}
_last_run = {}


def _build():
    from contextlib import ExitStack

    import concourse.bacc as bacc
    import concourse.mybir as mybir
    import concourse.tile as tile

    dt = mybir.dt
    AF = mybir.ActivationFunctionType
    ALU = mybir.AluOpType
    f32, bf16 = dt.float32, dt.bfloat16

    nc = bacc.Bacc("TRN2", target_bir_lowering=False, debug=False,
                   num_devices=NCORES)

    def din(name, shape, dty):
        return nc.dram_tensor(name, shape, dty, kind="ExternalInput").ap()

    xin_d = din("xin", [64, 2 * WA], bf16)
    wgt_d = din("wgt", [64, L * 3 * 32], bf16)   # tanh lhsT, 2 parity strips
    wgs_d = din("wgs", [64, L * 3 * 32], bf16)   # sigmoid lhsT
    wdx_d = din("wdx", [128, L * RD], bf16)      # dense lhsT (ring-strip rows)
    idw_d = din("idw", [128, 2 * RD], bf16)      # residual identity lhsT
    wskp_d = din("wskp", [128, NGRP * 4 * 128], bf16)  # skip lhsT per (grp, m)
    wp2_d = din("wp2", [128, 8 * 128], bf16)     # Wp2 lhsT per (q, p)
    btt_d = din("btt", [128, L], f32)            # bt stacked x4
    bst_d = din("bst", [128, L], f32)            # bs stacked x4
    bdc_d = din("bdc", [RD, L], f32)             # bdense
    hb_d = din("hb", [128, 4], f32)
    bp2c_d = din("bp2c", [128, 2], f32)
    sumw_d = din("sumw", [128, 1], f32)
    nones_d = din("nones", [1, 128], f32)
    out_d = nc.dram_tensor("out", [QD, V], f32, kind="ExternalOutput").ap()

    with tile.TileContext(nc) as tc, ExitStack() as top:
        wp = top.enter_context(tc.tile_pool(name="wp", bufs=1))

        def load(d, tag):
            t = wp.tile(list(d.shape), d.dtype, tag=tag, name=tag)
            nc.sync.dma_start(t[:], d[:])
            return t

        wgt = load(wgt_d, "wgt")
        wgs = load(wgs_d, "wgs")
        wdx = load(wdx_d, "wdx")
        idw = load(idw_d, "idw")
        wskp = load(wskp_d, "wskp")
        wp2 = load(wp2_d, "wp2")
        btt = load(btt_d, "btt")
        bst = load(bst_d, "bst")
        bdc = load(bdc_d, "bdc")
        hb = load(hb_d, "hb")
        bp2c = load(bp2c_d, "bp2c")
        sumw = load(sumw_d, "sumw")
        nones = load(nones_d, "nones")

        # x ping-pongs between partition strips 0/1 (32 each); rows 64:128
        # stay zero so K=128 identity-matmul contractions are exact.
        xx = wp.tile([128, WA], bf16, tag="xx", name="xx")
        ringA = wp.tile([128, WA], bf16, tag="ringA", name="ringA")
        ringB = wp.tile([128, WA], bf16, tag="ringB", name="ringB")
        rings = [ringA, ringB]
        # h layout: [128, cb*2048 + m*512] over the valid region
        h = wp.tile([128, 4 * VH], bf16, tag="h", name="h")
        nc.vector.memset(xx[:], 0.0)
        nc.vector.memset(ringA[:], 0.0)
        nc.vector.memset(ringB[:], 0.0)

        for half in range(2):
            nc.sync.dma_start(xx[0:64, :], xin_d[:, half * WA:(half + 1) * WA])
            with ExitStack() as lctx:
                pg = lctx.enter_context(
                    tc.tile_pool(name=f"pg{half}", bufs=2, space="PSUM"))
                pd = lctx.enter_context(
                    tc.tile_pool(name=f"pd{half}", bufs=2, space="PSUM"))
                tu = lctx.enter_context(tc.tile_pool(name=f"tu{half}", bufs=3))

                for l in range(L):
                    d = DIL[l]
                    j = l % 4
                    G = l // 4
                    ring = rings[G % 2]
                    sc = RD * (l % 2)        # strip of x_l
                    sn = RD * ((l + 1) % 2)  # strip of x_{l+1}
                    for u in range(NU):
                        b0 = PAD + u * 2048
                        pgt = pg.tile([128, 1024], f32, tag="pg", name="pg")
                        for s in range(4):
                            c0 = b0 + s * 512
                            for k in range(3):
                                nc.tensor.matmul(
                                    pgt[32 * s:32 * (s + 1), 0:512],
                                    wgt[sc:sc + RD,
                                        (l * 3 + k) * 32:(l * 3 + k + 1) * 32],
                                    xx[sc:sc + RD, c0 + (k - 1) * d:
                                       c0 + (k - 1) * d + 512],
                                    start=(k == 0), stop=(k == 2),
                                    tile_position=(sc, 32 * s))
                            for k in range(3):
                                nc.tensor.matmul(
                                    pgt[32 * s:32 * (s + 1), 512:1024],
                                    wgs[sc:sc + RD,
                                        (l * 3 + k) * 32:(l * 3 + k + 1) * 32],
                                    xx[sc:sc + RD, c0 + (k - 1) * d:
                                       c0 + (k - 1) * d + 512],
                                    start=(k == 0), stop=(k == 2),
                                    tile_position=(sc, 32 * s))
                        gt = tu.tile([128, 512], bf16, tag="gt", name="gt")
                        gs = tu.tile([128, 512], bf16, tag="gs", name="gs")
                        gm = tu.tile([128, 512], bf16, tag="gm", name="gm")
                        nc.scalar.activation(gt[:], pgt[:, 0:512], AF.Tanh,
                                             bias=btt[:, l:l + 1])
                        nc.scalar.activation(gs[:], pgt[:, 512:1024],
                                             AF.Sigmoid, bias=bst[:, l:l + 1])
                        nc.vector.tensor_mul(gm[:], gt[:], gs[:])
                        for s in range(4):
                            nc.vector.tensor_copy(
                                ring[RD * j:RD * (j + 1),
                                     b0 + 512 * s:b0 + 512 * (s + 1)],
                                gm[32 * s:32 * (s + 1), :])
                        # dense 1x1: two 1024-wide psum tiles per unit
                        for v2 in range(2):
                            c0 = b0 + v2 * 1024
                            pdt = pd.tile([RD, 1024], f32, tag="pd", name="pd")
                            for w2 in range(2):
                                nc.tensor.matmul(
                                    pdt[:, w2 * 512:(w2 + 1) * 512],
                                    wdx[:, l * RD:(l + 1) * RD],
                                    ring[:, c0 + w2 * 512:c0 + (w2 + 1) * 512],
                                    start=True, stop=(l % 2 == 0),
                                    tile_position=(0, 0))
                                if l % 2 == 1:
                                    # odd layers: residual via identity matmul
                                    nc.tensor.matmul(
                                        pdt[:, w2 * 512:(w2 + 1) * 512],
                                        idw[:, RD * (l % 2):RD * (l % 2) + RD],
                                        xx[:, c0 + w2 * 512:
                                           c0 + (w2 + 1) * 512],
                                        start=False, stop=True,
                                        tile_position=(0, 0))
                            if l % 2 == 0:
                                # x_new = (pd + bdense) + x_old on DVE
                                nc.vector.scalar_tensor_tensor(
                                    xx[sn:sn + RD, c0:c0 + 1024],
                                    pdt[:], bdc[:, l:l + 1],
                                    xx[sc:sc + RD, c0:c0 + 1024],
                                    op0=ALU.add, op1=ALU.add)
                            else:
                                # x_new = Identity(pd + bdense) on ScalarE
                                nc.scalar.activation(
                                    xx[sn:sn + RD, c0:c0 + 1024], pdt[:],
                                    AF.Identity, bias=bdc[:, l:l + 1])

                    if j == 3 or l == L - 1:
                        pidx = G // 2
                        if G % 2 == 1 or l == L - 1:
                            # skip contribution of this 8-layer pair
                            gA = 2 * pidx
                            grps = [g for g in (gA, gA + 1) if g <= G]
                            for m in range(4):
                                for cb in range(NB):
                                    c0 = VOFF + cb * 512
                                    pst = pg.tile([128, 512], f32, tag="pg",
                                                  name="pg")
                                    for gi, g in enumerate(grps):
                                        nc.tensor.matmul(
                                            pst[:],
                                            wskp[:, (g * 4 + m) * 128:
                                                 (g * 4 + m + 1) * 128],
                                            rings[g % 2][:, c0:c0 + 512],
                                            start=(gi == 0),
                                            stop=(gi == len(grps) - 1))
                                    hcol = cb * 2048 + m * 512
                                    if pidx == 0:
                                        # fold relu bias into h on first write
                                        nc.vector.tensor_scalar_add(
                                            h[:, hcol:hcol + 512], pst[:],
                                            hb[:, m:m + 1])
                                    else:
                                        nc.vector.tensor_add(
                                            h[:, hcol:hcol + 512],
                                            h[:, hcol:hcol + 512], pst[:])

            with ExitStack() as pctx:
                pop = pctx.enter_context(
                    tc.tile_pool(name=f"po{half}", bufs=4, space="PSUM"))
                psp = pctx.enter_context(
                    tc.tile_pool(name=f"ps{half}", bufs=2, space="PSUM"))
                sp = pctx.enter_context(tc.tile_pool(name=f"sp{half}", bufs=2))
                for cb in range(NB):
                    rr = sp.tile([128, 2048], bf16, tag="r", name="r")
                    nc.scalar.activation(rr[:], h[:, cb * 2048:(cb + 1) * 2048],
                                         AF.Relu)
                    pos = []
                    for p in range(2):
                        pot = pop.tile([128, 512], f32, tag="po", name="po")
                        for q in range(4):
                            nc.tensor.matmul(
                                pot[:],
                                wp2[:, (q * 2 + p) * 128:(q * 2 + p + 1) * 128],
                                rr[:, q * 512:(q + 1) * 512],
                                start=(q == 0), stop=(q == 3))
                        pos.append(pot)
                    ee = sp.tile([128, 1024], f32, tag="e", name="e")
                    for p in range(2):
                        nc.scalar.activation(ee[:, p * 512:(p + 1) * 512],
                                             pos[p][:], AF.Exp,
                                             bias=bp2c[:, p:p + 1])
                    pst = psp.tile([128, 512], f32, tag="ps", name="ps")
                    for p in range(2):
                        nc.tensor.matmul(pst[0:1, :], sumw[:],
                                         ee[:, p * 512:(p + 1) * 512],
                                         start=(p == 0), stop=(p == 1))
                    lss = sp.tile([1, 512], f32, tag="ls", name="ls")
                    nc.scalar.activation(lss[:], pst[0:1, :], AF.Ln)
                    pqt = psp.tile([128, 512], f32, tag="pq", name="pq")
                    nc.tensor.matmul(pqt[:], nones[:], lss[:],
                                     start=True, stop=True)
                    oo = sp.tile([128, 1024], f32, tag="o", name="o")
                    oo2 = sp.tile([128, 1024], f32, tag="o2", name="o2")
                    for p in range(2):
                        # oo = pos + bp2 (ScalarE, psum read), then
                        # oo2 = oo + (-log sumexp broadcast) (DVE, 1 psum op)
                        nc.scalar.activation(oo[:, p * 512:(p + 1) * 512],
                                             pos[p][:], AF.Identity,
                                             bias=bp2c[:, p:p + 1])
                        nc.vector.scalar_tensor_tensor(
                            oo2[:, p * 512:(p + 1) * 512],
                            pqt[:], 0.0,
                            oo[:, p * 512:(p + 1) * 512],
                            op0=ALU.add, op1=ALU.add)
                        c0 = half * VH + cb * 512
                        nc.sync.dma_start(
                            out_d[p * 128:(p + 1) * 128, c0:c0 + 512],
                            oo2[:, p * 512:(p + 1) * 512])

    nc.compile()
    return nc


def _prep_host(inputs):
    """Host-side exact fp32 preprocessing: initial conv, weight packing."""
    x = np.asarray(inputs["x"], np.float32)
    Wc = np.asarray(inputs["Wc"], np.float32)
    bc = np.asarray(inputs["bc"], np.float32)
    Wt = np.asarray(inputs["Wt"], np.float32)
    bt = np.asarray(inputs["bt"], np.float32)
    Ws = np.asarray(inputs["Ws"], np.float32)
    bs = np.asarray(inputs["bs"], np.float32)
    Wskip = np.asarray(inputs["Wskip"], np.float32)
    bskip = np.asarray(inputs["bskip"], np.float32)
    Wdense = np.asarray(inputs["Wdense"], np.float32)
    bdense = np.asarray(inputs["bdense"], np.float32)
    Wp1 = np.asarray(inputs["Wp1"], np.float32)
    bp1 = np.asarray(inputs["bp1"], np.float32)
    Wp2 = np.asarray(inputs["Wp2"], np.float32)
    bp2 = np.asarray(inputs["bp2"], np.float32)

    # initial conv (1 -> 32, k=3, pad=1), exact fp32 on host
    x0 = x[0, 0]
    xp = np.pad(x0, (1, 1))
    x1 = (Wc[:, 0, 0:1] * xp[None, 0:T]
          + Wc[:, 0, 1:2] * xp[None, 1:T + 1]
          + Wc[:, 0, 2:3] * xp[None, 2:T + 2]) + bc[:, None]
    xg = np.pad(x1, ((0, 0), (VOFF, VOFF)))

    xin = np.zeros((NCORES, 64, 2 * WA), BF16)
    for c in range(NCORES):
        for hf in range(2):
            s = c * V + hf * VH
            xin[c, 0:RD, hf * WA:(hf + 1) * WA] = xg[:, s:s + WA].astype(BF16)

    wgt = np.zeros((64, L * 3 * 32), np.float32)
    wgs = np.zeros((64, L * 3 * 32), np.float32)
    for l in range(L):
        for k in range(3):
            for p in range(2):
                wgt[RD * p:RD * (p + 1),
                    (l * 3 + k) * 32:(l * 3 + k + 1) * 32] = Wt[l, :, :, k].T
                wgs[RD * p:RD * (p + 1),
                    (l * 3 + k) * 32:(l * 3 + k + 1) * 32] = Ws[l, :, :, k].T

    wdx = np.zeros((128, L * RD), np.float32)
    for l in range(L):
        jj = l % 4
        wdx[RD * jj:RD * (jj + 1), l * RD:(l + 1) * RD] = Wdense[l, :, :, 0].T

    idw = np.zeros((128, 2 * RD), np.float32)
    for p in range(2):
        idw[RD * p:RD * (p + 1), RD * p:RD * (p + 1)] = np.eye(RD)

    W1s = np.einsum("ab,lbc->lac", Wp1[:, :, 0], Wskip[:, :, :, 0])  # [L,512,32]
    wskp = np.zeros((128, NGRP * 4 * 128), np.float32)
    for G in range(NGRP):
        for m in range(4):
            for jj in range(4):
                l = G * 4 + jj
                if l < L:
                    wskp[32 * jj:32 * (jj + 1),
                         (G * 4 + m) * 128:(G * 4 + m + 1) * 128] = \
                        W1s[l, 128 * m:128 * (m + 1), :].T

    wp2 = np.zeros((128, 8 * 128), np.float32)
    for q in range(4):
        for p in range(2):
            wp2[:, (q * 2 + p) * 128:(q * 2 + p + 1) * 128] = \
                Wp2[128 * p:128 * (p + 1), 128 * q:128 * (q + 1), 0].T

    hbias = Wp1[:, :, 0] @ bskip.sum(axis=0) + bp1     # [512]
    hb = hbias.reshape(4, 128).T.copy()                # [128, 4]

    shared = {
        "wgt": wgt.astype(BF16),
        "wgs": wgs.astype(BF16),
        "wdx": wdx.astype(BF16),
        "idw": idw.astype(BF16),
        "wskp": wskp.astype(BF16),
        "wp2": wp2.astype(BF16),
        "btt": np.ascontiguousarray(np.tile(bt.T, (4, 1)).astype(np.float32)),
        "bst": np.ascontiguousarray(np.tile(bs.T, (4, 1)).astype(np.float32)),
        "bdc": np.ascontiguousarray(bdense.T.astype(np.float32)),
        "hb": np.ascontiguousarray(hb.astype(np.float32)),
        "bp2c": np.ascontiguousarray(bp2.reshape(2, 128).T.astype(np.float32)),
        "sumw": np.ones((128, 1), np.float32),
        "nones": np.full((1, 128), -1.0, np.float32),
    }
    return xin, shared


def kernel(**inputs):
    from concourse.bass_utils import run_bass_kernel_spmd

    xin, shared = _prep_host(inputs)
    if "nc" not in _cache:
        _cache["nc"] = _build()
    nc = _cache["nc"]

    in_maps = [dict(shared, xin=np.ascontiguousarray(xin[c]))
               for c in range(NCORES)]
    res = run_bass_kernel_spmd(nc, in_maps, core_ids=list(range(NCORES)))

    _last_run["nc"] = nc
    _last_run["in_maps"] = in_maps

    out = np.empty((1, QD, T), np.float32)
    for c in range(NCORES):
        out[0, :, c * V:(c + 1) * V] = res.results[c]["out"]
    return out
